# revision 2
# baseline (speedup 1.0000x reference)
"""Fused 2-layer KAN for Trainium2, data-parallel across 8 NeuronCores.

Math: with G=3 grid points the spline basis is piecewise-linear in x, so each
KAN layer collapses to a small dense matmul over 3 cheap feature maps:

    out = bias + silu(x) @ Wb + u @ P1 + C @ (P2 - P1)
      u = clip(x, -1, 1),  C = max(u, 0)
      Wb = imp*bw;  T = imp*sw*cp;  P1 = T@(bv1-bv0);  P2 = T@(bv2-bv1)
      bias_j = sum_i T[i,j,:] @ bv1

All K=5 spline control points fold into P1/P2/bias on the host (O(I*J*K) work).
The device runs, per 1024-row macro-tile:
  DMA in -> PE transpose x to feature-major -> {silu, clip} feature maps
  -> L1 matmul (bf16, N=512, two 64-contraction row-group streams)
  -> L2 feature maps from PSUM -> L2 matmul (stationary features, N=64)
  -> bias via K=1 ones-matmul PSUM init -> copy to SBUF -> DMA out.
"""

import os
import sys
from contextlib import ExitStack

import numpy as np
import ml_dtypes

for _p in ("/opt/trn_rl_repo",):
    if _p not in sys.path and os.path.isdir(_p):
        sys.path.insert(0, _p)

import concourse.bass as bass
import concourse.tile as tile
from concourse import bacc, mybir
from concourse.bass_utils import run_bass_kernel_spmd
from concourse.masks import make_identity

F32 = mybir.dt.float32
BF16 = mybir.dt.bfloat16
BF = ml_dtypes.bfloat16

N_CORES = 8
D0, D1, D2 = 64, 128, 64
K, DEG, G, LO, HI = 5, 3, 3, -1.0, 1.0
MACRO = 1024  # batch rows per device macro-iteration

_nc_cache = {}


def _basis_table():
    knots = np.linspace(LO - DEG * 0.1, HI + DEG * 0.1, K + DEG + 1)
    grid = np.linspace(LO, HI, G)
    bv = np.zeros((G, K), dtype=np.float32)
    for i in range(K):
        center = (knots[i + DEG // 2] + knots[i + DEG // 2 + 1]) / 2.0
        width = (knots[i + DEG + 1] - knots[i]) / 2.0
        bv[:, i] = np.exp(-(((grid - center) / width) ** 2))
    bv = bv / (bv.sum(axis=1, keepdims=True) + 1e-6)
    return bv


def _prep_consts(cp0, bw0, sw0, imp0, cp1, bw1, sw1, imp1):
    f8 = np.float64
    bv = _basis_table().astype(f8)
    d1, d2 = bv[1] - bv[0], bv[2] - bv[1]

    def fold(cp, bw, sw, imp):
        T = imp.astype(f8)[:, :, None] * sw.astype(f8)[:, :, None] * cp.astype(f8)
        Wb = imp.astype(f8) * bw.astype(f8)
        return Wb, T @ d1, T @ d2, (T @ bv[1]).sum(axis=0)

    Wb0, P10, P20, b1 = fold(cp0, bw0, sw0, imp0)
    Wb1, P11, P21, b2 = fold(cp1, bw1, sw1, imp1)
    bias2_eff = b2 + b1 @ P21

    w1 = np.stack([Wb0, P10, P20 - P10], axis=0)  # [3, 64, 128] lhsT chunks
    w1 = np.concatenate([w1, w1], axis=1)  # duplicate rows for partitions 64-127
    w1 = np.ascontiguousarray(w1.transpose(1, 0, 2)).astype(BF)  # [128, 3, 128]
    w2 = np.stack([Wb1, P11, P21 - P11], axis=0)  # [3, 128, 64] rhs chunks
    w2 = np.ascontiguousarray(w2.transpose(1, 0, 2)).astype(BF)  # [128, 3, 64]

    return {
        "w1": w1,
        "w2": w2,
        "b1": b1.astype(np.float32).reshape(D1, 1),
        "s1": (-1.0 - b1).astype(np.float32).reshape(D1, 1),
        "s2": (1.0 - b1).astype(np.float32).reshape(D1, 1),
        "nb1": (-b1).astype(np.float32).reshape(D1, 1),
        "b2row": np.tile(bias2_eff, 8).astype(BF).reshape(1, 512),
    }


def _build(rows):
    assert rows % MACRO == 0
    nc = bacc.Bacc(
        "TRN2",
        target_bir_lowering=False,
        debug=False,
        enable_asserts=False,
        num_devices=N_CORES,
    )
    xd = nc.dram_tensor("x", [rows, D0], F32, kind="ExternalInput")
    w1d = nc.dram_tensor("w1", [128, 3, 128], BF16, kind="ExternalInput")
    w2d = nc.dram_tensor("w2", [128, 3, 64], BF16, kind="ExternalInput")
    b1d = nc.dram_tensor("b1", [D1, 1], F32, kind="ExternalInput")
    s1d = nc.dram_tensor("s1", [D1, 1], F32, kind="ExternalInput")
    s2d = nc.dram_tensor("s2", [D1, 1], F32, kind="ExternalInput")
    nb1d = nc.dram_tensor("nb1", [D1, 1], F32, kind="ExternalInput")
    b2d = nc.dram_tensor("b2row", [1, 512], BF16, kind="ExternalInput")
    outd = nc.dram_tensor("out", [rows, D2], F32, kind="ExternalOutput")

    n_macro = rows // MACRO
    MAX, MIN = mybir.AluOpType.max, mybir.AluOpType.min
    SILU = mybir.ActivationFunctionType.Silu

    with tile.TileContext(nc) as tc, ExitStack() as ctx:
        consts = ctx.enter_context(tc.tile_pool(name="consts", bufs=1))
        xin = ctx.enter_context(tc.tile_pool(name="xin", bufs=3))
        f1 = ctx.enter_context(tc.tile_pool(name="f1", bufs=2))
        f2 = ctx.enter_context(tc.tile_pool(name="f2", bufs=2))
        osb = ctx.enter_context(tc.tile_pool(name="osb", bufs=3))
        ps_x = ctx.enter_context(tc.tile_pool(name="ps_x", bufs=2, space="PSUM"))
        ps_h = ctx.enter_context(tc.tile_pool(name="ps_h", bufs=2, space="PSUM"))
        ps_o = ctx.enter_context(tc.tile_pool(name="ps_o", bufs=2, space="PSUM"))

        ident = consts.tile([128, 128], F32)
        make_identity(nc, ident)
        ones = consts.tile([1, 128], BF16)
        nc.vector.memset(ones, 1.0)
        w1 = consts.tile([128, 3, 128], BF16)
        nc.sync.dma_start(w1, w1d.ap())
        w2 = consts.tile([128, 3, 64], BF16)
        nc.sync.dma_start(w2, w2d.ap())
        b1 = consts.tile([D1, 1], F32)
        nc.sync.dma_start(b1, b1d.ap())
        s1 = consts.tile([D1, 1], F32)
        nc.sync.dma_start(s1, s1d.ap())
        s2 = consts.tile([D1, 1], F32)
        nc.sync.dma_start(s2, s2d.ap())
        nb1 = consts.tile([D1, 1], F32)
        nc.sync.dma_start(nb1, nb1d.ap())
        b2r = consts.tile([1, 512], BF16)
        nc.sync.dma_start(b2r, b2d.ap())

        for m in range(n_macro):
            base = m * MACRO
            # x[base + (2q+j)*128 + p, f] -> xt[p, q, j, f]
            xt = xin.tile([128, 4, 2, 64], F32, tag="xt")
            src = bass.AP(
                xd, base * 64, [[64, 128], [2 * 128 * 64, 4], [128 * 64, 2], [1, 64]]
            )
            nc.sync.dma_start(xt, src)

            # transpose: px[p,q,:] partitions 0-63 = feats of block 2q,
            # partitions 64-127 = feats of block 2q+1; free = 128 rows
            px = ps_x.tile([128, 4, 128], F32, tag="px")
            for q in range(4):
                nc.tensor.transpose(px[:, q], xt[:, q], ident)

            u1 = f1.tile([128, 4, 128], BF16, tag="u1")
            nc.vector.tensor_scalar(u1, px, -1.0, 1.0, op0=MAX, op1=MIN)
            sl1 = f1.tile([128, 4, 128], BF16, tag="sl1")
            nc.scalar.activation(sl1, px, SILU)
            c1 = f1.tile([128, 4, 128], BF16, tag="c1")
            nc.gpsimd.tensor_scalar_max(c1, u1, 0.0)

            # L1: two concurrent 64-contraction row-group streams (A=even
            # blocks on partitions 0-63, B=odd blocks on 64-127)
            hA = ps_h.tile([128, 512], F32, tag="hA")
            hB = ps_h.tile([128, 512], F32, tag="hB")
            for c, ft in enumerate([sl1, u1, c1]):
                nc.tensor.matmul(hA, w1[0:64, c], ft[0:64], start=(c == 0), stop=(c == 2))
                nc.tensor.matmul(hB, w1[64:128, c], ft[64:128], start=(c == 0), stop=(c == 2))

            l2 = []
            for half, h in enumerate([hA, hB]):
                slh = f2.tile([128, 512], BF16, tag=f"sl2{half}")
                nc.scalar.activation(slh, h, SILU, bias=b1)
                uh = f2.tile([128, 512], BF16, tag=f"u2{half}")
                nc.vector.tensor_scalar(uh, h, s1, s2, op0=MAX, op1=MIN)
                ch = f2.tile([128, 512], BF16, tag=f"c2{half}")
                nc.gpsimd.tensor_scalar_max(ch, uh, nb1)
                l2.append((slh, uh, ch))

            # L2: bias init via K=1 ones-matmul (sets has_written on the whole
            # bank so the 24 block matmuls accumulate with start=False)
            po = ps_o.tile([128, 8, 64], F32, tag="po")
            nc.tensor.matmul(po, ones, b2r, start=True, stop=False)
            for k in range(4):
                for half in range(2):
                    slh, uh, ch = l2[half]
                    g = 2 * k + half
                    for c, ft in enumerate([slh, uh, ch]):
                        nc.tensor.matmul(
                            po[:, g],
                            ft[:, k * 128 : (k + 1) * 128],
                            w2[:, c],
                            start=False,
                            stop=(g == 7 and c == 2),
                        )

            ot = osb.tile([128, 8, 64], F32, tag="ot")
            if m % 2 == 0:
                nc.vector.tensor_copy(ot, po)
            else:
                nc.scalar.copy(ot, po)
            dst = bass.AP(outd, base * 64, [[64, 128], [128 * 64, 8], [1, 64]])
            nc.sync.dma_start(dst, ot)

    nc.compile()
    return nc


def _get_nc(rows):
    if rows not in _nc_cache:
        _nc_cache[rows] = _build(rows)
    return _nc_cache[rows]


def kernel(x, cp0, bw0, sw0, imp0, cp1, bw1, sw1, imp1, _trace=False, _trace_kwargs=None):
    x = np.ascontiguousarray(np.asarray(x, dtype=np.float32))
    consts = _prep_consts(
        *[np.asarray(a, dtype=np.float32) for a in (cp0, bw0, sw0, imp0, cp1, bw1, sw1, imp1)]
    )
    rows = x.shape[0] // N_CORES
    nc = _get_nc(rows)
    in_maps = []
    for i in range(N_CORES):
        m = dict(consts)
        m["x"] = x[i * rows : (i + 1) * rows]
        in_maps.append(m)
    res = run_bass_kernel_spmd(
        nc, in_maps, list(range(N_CORES)), trace=_trace, **(_trace_kwargs or {})
    )
    out = np.concatenate([res.results[i]["out"] for i in range(N_CORES)], axis=0)
    if _trace:
        return out, res
    return out


# revision 5
# speedup vs baseline: 3.6201x; 3.6201x over previous
"""Fused 2-layer KAN for Trainium2, data-parallel across 8 NeuronCores.

Math: with G=3 grid points the spline basis is piecewise-linear in x, so each
KAN layer collapses to a small dense matmul over 3 cheap feature maps:

    out = bias + silu(x) @ Wb + u @ P1 + C @ (P2 - P1)
      u = clip(x, -1, 1),  C = max(u, 0)
      Wb = imp*bw;  T = imp*sw*cp;  P1 = T@(bv1-bv0);  P2 = T@(bv2-bv1)
      bias_j = sum_i T[i,j,:] @ bv1

All K=5 spline control points fold into P1/P2/bias on the host (O(I*J*K) work).
The device runs, per 1024-row macro-tile:
  DMA in -> PE transpose x to feature-major -> {silu, clip} feature maps
  -> L1 matmul (bf16, N=512, two 64-contraction row-group streams)
  -> L2 feature maps from PSUM -> L2 matmul (stationary features, N=64)
  -> bias via K=1 ones-matmul PSUM init -> copy to SBUF -> DMA out.
"""

import os
import sys
from contextlib import ExitStack

import numpy as np
import ml_dtypes

for _p in ("/opt/trn_rl_repo",):
    if _p not in sys.path and os.path.isdir(_p):
        sys.path.insert(0, _p)

import concourse.bass as bass
import concourse.tile as tile
from concourse import bacc, mybir
from concourse.bass_utils import run_bass_kernel_spmd
from concourse.masks import make_identity

F32 = mybir.dt.float32
BF16 = mybir.dt.bfloat16
BF = ml_dtypes.bfloat16

N_CORES = 8
D0, D1, D2 = 64, 128, 64
K, DEG, G, LO, HI = 5, 3, 3, -1.0, 1.0
MACRO = 1024  # batch rows per device macro-iteration

_nc_cache = {}


def _basis_table():
    knots = np.linspace(LO - DEG * 0.1, HI + DEG * 0.1, K + DEG + 1)
    grid = np.linspace(LO, HI, G)
    bv = np.zeros((G, K), dtype=np.float32)
    for i in range(K):
        center = (knots[i + DEG // 2] + knots[i + DEG // 2 + 1]) / 2.0
        width = (knots[i + DEG + 1] - knots[i]) / 2.0
        bv[:, i] = np.exp(-(((grid - center) / width) ** 2))
    bv = bv / (bv.sum(axis=1, keepdims=True) + 1e-6)
    return bv


def _prep_consts(cp0, bw0, sw0, imp0, cp1, bw1, sw1, imp1):
    f8 = np.float64
    bv = _basis_table().astype(f8)
    d1, d2 = bv[1] - bv[0], bv[2] - bv[1]

    def fold(cp, bw, sw, imp):
        T = imp.astype(f8)[:, :, None] * sw.astype(f8)[:, :, None] * cp.astype(f8)
        Wb = imp.astype(f8) * bw.astype(f8)
        return Wb, T @ d1, T @ d2, (T @ bv[1]).sum(axis=0)

    Wb0, P10, P20, b1 = fold(cp0, bw0, sw0, imp0)
    Wb1, P11, P21, b2 = fold(cp1, bw1, sw1, imp1)
    bias2_eff = b2 + b1 @ P21

    w1 = np.stack([Wb0, P10, P20 - P10], axis=0)  # [3, 64, 128] lhsT chunks
    w1 = np.concatenate([w1, w1], axis=1)  # duplicate rows for partitions 64-127
    w1 = np.ascontiguousarray(w1.transpose(1, 0, 2)).astype(BF)  # [128, 3, 128]
    w2 = np.stack([Wb1, P11, P21 - P11], axis=0)  # [3, 128, 64] rhs chunks
    w2 = np.ascontiguousarray(w2.transpose(1, 0, 2)).astype(BF)  # [128, 3, 64]

    return {
        "w1": w1,
        "w2": w2,
        "b1": b1.astype(np.float32).reshape(D1, 1),
        "s1": (-1.0 - b1).astype(np.float32).reshape(D1, 1),
        "s2": (1.0 - b1).astype(np.float32).reshape(D1, 1),
        "nb1": (-b1).astype(np.float32).reshape(D1, 1),
        "b2row": np.tile(bias2_eff, 8).astype(BF).reshape(1, 512),
    }


def _build(rows):
    assert rows % MACRO == 0
    nc = bacc.Bacc(
        "TRN2",
        target_bir_lowering=False,
        debug=False,
        enable_asserts=False,
        num_devices=N_CORES,
    )
    xd = nc.dram_tensor("x", [rows, D0], F32, kind="ExternalInput")
    w1d = nc.dram_tensor("w1", [128, 3, 128], BF16, kind="ExternalInput")
    w2d = nc.dram_tensor("w2", [128, 3, 64], BF16, kind="ExternalInput")
    b1d = nc.dram_tensor("b1", [D1, 1], F32, kind="ExternalInput")
    s1d = nc.dram_tensor("s1", [D1, 1], F32, kind="ExternalInput")
    s2d = nc.dram_tensor("s2", [D1, 1], F32, kind="ExternalInput")
    nb1d = nc.dram_tensor("nb1", [D1, 1], F32, kind="ExternalInput")
    b2d = nc.dram_tensor("b2row", [1, 512], BF16, kind="ExternalInput")
    outd = nc.dram_tensor("out", [rows, D2], F32, kind="ExternalOutput")

    n_macro = rows // MACRO
    MAX, MIN = mybir.AluOpType.max, mybir.AluOpType.min
    SILU = mybir.ActivationFunctionType.Silu

    with tile.TileContext(nc) as tc, ExitStack() as ctx:
        consts = ctx.enter_context(tc.tile_pool(name="consts", bufs=1))
        xin = ctx.enter_context(tc.tile_pool(name="xin", bufs=3))
        f1 = ctx.enter_context(tc.tile_pool(name="f1", bufs=2))
        f2 = ctx.enter_context(tc.tile_pool(name="f2", bufs=2))
        osb = ctx.enter_context(tc.tile_pool(name="osb", bufs=3))
        ps_x = ctx.enter_context(tc.tile_pool(name="ps_x", bufs=2, space="PSUM"))
        ps_h = ctx.enter_context(tc.tile_pool(name="ps_h", bufs=2, space="PSUM"))
        ps_o = ctx.enter_context(tc.tile_pool(name="ps_o", bufs=2, space="PSUM"))

        ident = consts.tile([128, 128], F32)
        make_identity(nc, ident)
        ones = consts.tile([1, 128], BF16)
        nc.vector.memset(ones, 1.0)
        w1 = consts.tile([128, 3, 128], BF16)
        nc.sync.dma_start(w1, w1d.ap())
        w2 = consts.tile([128, 3, 64], BF16)
        nc.sync.dma_start(w2, w2d.ap())
        b1 = consts.tile([D1, 1], F32)
        nc.sync.dma_start(b1, b1d.ap())
        s1 = consts.tile([D1, 1], F32)
        nc.sync.dma_start(s1, s1d.ap())
        s2 = consts.tile([D1, 1], F32)
        nc.sync.dma_start(s2, s2d.ap())
        nb1 = consts.tile([D1, 1], F32)
        nc.sync.dma_start(nb1, nb1d.ap())
        b2r = consts.tile([1, 512], BF16)
        nc.sync.dma_start(b2r, b2d.ap())

        for m in range(n_macro):
            base = m * MACRO
            # x[base + (2q+j)*128 + p, f] -> xt[p, q, j, f]
            xt = xin.tile([128, 4, 2, 64], F32, tag="xt")
            src = bass.AP(
                xd, base * 64, [[64, 128], [2 * 128 * 64, 4], [128 * 64, 2], [1, 64]]
            )
            nc.sync.dma_start(xt, src)

            # transpose: px[p,q,:] partitions 0-63 = feats of block 2q,
            # partitions 64-127 = feats of block 2q+1; free = 128 rows
            px = ps_x.tile([128, 4, 128], F32, tag="px")
            for q in range(4):
                nc.tensor.transpose(px[:, q], xt[:, q], ident)

            u1 = f1.tile([128, 4, 128], BF16, tag="u1")
            nc.vector.tensor_scalar(u1, px, -1.0, 1.0, op0=MAX, op1=MIN)
            sl1 = f1.tile([128, 4, 128], BF16, tag="sl1")
            nc.scalar.activation(sl1, px, SILU)
            c1 = f1.tile([128, 4, 128], BF16, tag="c1")
            nc.vector.tensor_scalar_max(c1, u1, 0.0)

            # L1: two concurrent 64-contraction row-group streams (A=even
            # blocks on partitions 0-63, B=odd blocks on 64-127)
            hA = ps_h.tile([128, 512], F32, tag="hA")
            hB = ps_h.tile([128, 512], F32, tag="hB")
            for c, ft in enumerate([sl1, u1, c1]):
                nc.tensor.matmul(hA, w1[0:64, c], ft[0:64], start=(c == 0), stop=(c == 2))
                nc.tensor.matmul(hB, w1[64:128, c], ft[64:128], start=(c == 0), stop=(c == 2))

            l2 = []
            for half, h in enumerate([hA, hB]):
                slh = f2.tile([128, 512], BF16, tag=f"sl2{half}")
                nc.scalar.activation(slh, h, SILU, bias=b1)
                uh = f2.tile([128, 512], BF16, tag=f"u2{half}")
                nc.vector.tensor_scalar(uh, h, s1, s2, op0=MAX, op1=MIN)
                ch = f2.tile([128, 512], BF16, tag=f"c2{half}")
                nc.vector.tensor_scalar_max(ch, uh, nb1)
                l2.append((slh, uh, ch))

            # L2: bias init via K=1 ones-matmul (sets has_written on the whole
            # bank so the 24 block matmuls accumulate with start=False)
            po = ps_o.tile([128, 8, 64], F32, tag="po")
            nc.tensor.matmul(po, ones, b2r, start=True, stop=False)
            for k in range(4):
                for half in range(2):
                    slh, uh, ch = l2[half]
                    g = 2 * k + half
                    for c, ft in enumerate([slh, uh, ch]):
                        nc.tensor.matmul(
                            po[:, g],
                            ft[:, k * 128 : (k + 1) * 128],
                            w2[:, c],
                            start=False,
                            stop=(g == 7 and c == 2),
                        )

            ot = osb.tile([128, 8, 64], F32, tag="ot")
            nc.scalar.copy(ot, po)
            dst = bass.AP(outd, base * 64, [[64, 128], [128 * 64, 8], [1, 64]])
            nc.sync.dma_start(dst, ot)

    nc.compile()
    return nc


def _get_nc(rows):
    if rows not in _nc_cache:
        _nc_cache[rows] = _build(rows)
    return _nc_cache[rows]


def kernel(x, cp0, bw0, sw0, imp0, cp1, bw1, sw1, imp1, _trace=False, _trace_kwargs=None):
    x = np.ascontiguousarray(np.asarray(x, dtype=np.float32))
    consts = _prep_consts(
        *[np.asarray(a, dtype=np.float32) for a in (cp0, bw0, sw0, imp0, cp1, bw1, sw1, imp1)]
    )
    rows = x.shape[0] // N_CORES
    nc = _get_nc(rows)
    in_maps = []
    for i in range(N_CORES):
        m = dict(consts)
        m["x"] = x[i * rows : (i + 1) * rows]
        in_maps.append(m)
    res = run_bass_kernel_spmd(
        nc, in_maps, list(range(N_CORES)), trace=_trace, **(_trace_kwargs or {})
    )
    out = np.concatenate([res.results[i]["out"] for i in range(N_CORES)], axis=0)
    if _trace:
        return out, res
    return out


# revision 7
# speedup vs baseline: 4.1717x; 1.1524x over previous
"""Fused 2-layer KAN for Trainium2, data-parallel across 8 NeuronCores.

Math: with G=3 grid points the spline basis is piecewise-linear in x, so each
KAN layer collapses to a small dense matmul over 3 cheap feature maps:

    out = bias + silu(x) @ Wb + u @ P1 + C @ (P2 - P1)
      u = clip(x, -1, 1),  C = max(u, 0)
      Wb = imp*bw;  T = imp*sw*cp;  P1 = T@(bv1-bv0);  P2 = T@(bv2-bv1)
      bias_j = sum_i T[i,j,:] @ bv1

All K=5 spline control points fold into P1/P2/bias on the host (O(I*J*K) work).
The device runs, per 1024-row macro-tile:
  DMA in -> PE transpose x to feature-major -> {silu, clip} feature maps
  -> L1 matmul (bf16, N=512, two 64-contraction row-group streams)
  -> L2 feature maps from PSUM -> L2 matmul (stationary features, N=64)
  -> bias via K=1 ones-matmul PSUM init -> copy to SBUF -> DMA out.
"""

import os
import sys
from contextlib import ExitStack

import numpy as np
import ml_dtypes

for _p in ("/opt/trn_rl_repo",):
    if _p not in sys.path and os.path.isdir(_p):
        sys.path.insert(0, _p)

import concourse.bass as bass
import concourse.tile as tile
from concourse import bacc, mybir
from concourse.bass_utils import run_bass_kernel_spmd
from concourse.masks import make_identity

F32 = mybir.dt.float32
BF16 = mybir.dt.bfloat16
BF = ml_dtypes.bfloat16

N_CORES = 8
D0, D1, D2 = 64, 128, 64
K, DEG, G, LO, HI = 5, 3, 3, -1.0, 1.0
MACRO = 1024  # batch rows per device macro-iteration

_nc_cache = {}


def _basis_table():
    knots = np.linspace(LO - DEG * 0.1, HI + DEG * 0.1, K + DEG + 1)
    grid = np.linspace(LO, HI, G)
    bv = np.zeros((G, K), dtype=np.float32)
    for i in range(K):
        center = (knots[i + DEG // 2] + knots[i + DEG // 2 + 1]) / 2.0
        width = (knots[i + DEG + 1] - knots[i]) / 2.0
        bv[:, i] = np.exp(-(((grid - center) / width) ** 2))
    bv = bv / (bv.sum(axis=1, keepdims=True) + 1e-6)
    return bv


def _prep_consts(cp0, bw0, sw0, imp0, cp1, bw1, sw1, imp1):
    f8 = np.float64
    bv = _basis_table().astype(f8)
    d1, d2 = bv[1] - bv[0], bv[2] - bv[1]

    def fold(cp, bw, sw, imp):
        T = imp.astype(f8)[:, :, None] * sw.astype(f8)[:, :, None] * cp.astype(f8)
        Wb = imp.astype(f8) * bw.astype(f8)
        return Wb, T @ d1, T @ d2, (T @ bv[1]).sum(axis=0)

    Wb0, P10, P20, b1 = fold(cp0, bw0, sw0, imp0)
    Wb1, P11, P21, b2 = fold(cp1, bw1, sw1, imp1)
    bias2_eff = b2 + b1 @ P21

    w1 = np.stack([Wb0, P10, P20 - P10], axis=0)  # [3, 64, 128] lhsT chunks
    w1 = np.concatenate([w1, w1], axis=1)  # duplicate rows for partitions 64-127
    w1 = np.ascontiguousarray(w1.transpose(1, 0, 2)).astype(BF)  # [128, 3, 128]
    w2 = np.stack([Wb1, P11, P21 - P11], axis=0)  # [3, 128, 64] rhs chunks
    w2 = np.ascontiguousarray(w2.transpose(1, 0, 2)).astype(BF)  # [128, 3, 64]

    return {
        "w1": w1,
        "w2": w2,
        "b1": b1.astype(np.float32).reshape(D1, 1),
        "s1": (-1.0 - b1).astype(np.float32).reshape(D1, 1),
        "s2": (1.0 - b1).astype(np.float32).reshape(D1, 1),
        "nb1": (-b1).astype(np.float32).reshape(D1, 1),
        "b2row": np.tile(bias2_eff, 8).astype(BF).reshape(1, 512),
    }


def _build(rows):
    assert rows % MACRO == 0
    nc = bacc.Bacc(
        "TRN2",
        target_bir_lowering=False,
        debug=False,
        enable_asserts=False,
        num_devices=N_CORES,
    )
    xd = nc.dram_tensor("x", [rows, D0], F32, kind="ExternalInput")
    w1d = nc.dram_tensor("w1", [128, 3, 128], BF16, kind="ExternalInput")
    w2d = nc.dram_tensor("w2", [128, 3, 64], BF16, kind="ExternalInput")
    b1d = nc.dram_tensor("b1", [D1, 1], F32, kind="ExternalInput")
    s1d = nc.dram_tensor("s1", [D1, 1], F32, kind="ExternalInput")
    s2d = nc.dram_tensor("s2", [D1, 1], F32, kind="ExternalInput")
    nb1d = nc.dram_tensor("nb1", [D1, 1], F32, kind="ExternalInput")
    b2d = nc.dram_tensor("b2row", [1, 512], BF16, kind="ExternalInput")
    outd = nc.dram_tensor("out", [rows, D2], F32, kind="ExternalOutput")

    n_macro = rows // MACRO
    MAX, MIN = mybir.AluOpType.max, mybir.AluOpType.min
    SILU = mybir.ActivationFunctionType.Silu

    with tile.TileContext(nc) as tc, ExitStack() as ctx:
        consts = ctx.enter_context(tc.tile_pool(name="consts", bufs=1))
        xin = ctx.enter_context(tc.tile_pool(name="xin", bufs=4))
        f1 = ctx.enter_context(tc.tile_pool(name="f1", bufs=3))
        f2 = ctx.enter_context(tc.tile_pool(name="f2", bufs=3))
        osb = ctx.enter_context(tc.tile_pool(name="osb", bufs=3))
        ps_x = ctx.enter_context(tc.tile_pool(name="ps_x", bufs=2, space="PSUM"))
        ps_h = ctx.enter_context(tc.tile_pool(name="ps_h", bufs=2, space="PSUM"))
        ps_o = ctx.enter_context(tc.tile_pool(name="ps_o", bufs=2, space="PSUM"))

        ident = consts.tile([128, 128], F32)
        make_identity(nc, ident)
        ones = consts.tile([1, 128], BF16)
        nc.vector.memset(ones, 1.0)
        w1 = consts.tile([128, 3, 128], BF16)
        nc.sync.dma_start(w1, w1d.ap())
        w2 = consts.tile([128, 3, 64], BF16)
        nc.sync.dma_start(w2, w2d.ap())
        b1 = consts.tile([D1, 1], F32)
        nc.sync.dma_start(b1, b1d.ap())
        s1 = consts.tile([D1, 1], F32)
        nc.sync.dma_start(s1, s1d.ap())
        s2 = consts.tile([D1, 1], F32)
        nc.sync.dma_start(s2, s2d.ap())
        nb1 = consts.tile([D1, 1], F32)
        nc.sync.dma_start(nb1, nb1d.ap())
        b2r = consts.tile([1, 512], BF16)
        nc.sync.dma_start(b2r, b2d.ap())

        for m in range(n_macro):
            base = m * MACRO
            # x[base + (2q+j)*128 + p, f] -> xt[p, q, j, f]
            xt = xin.tile([128, 4, 2, 64], F32, tag="xt")
            src = bass.AP(
                xd, base * 64, [[64, 128], [2 * 128 * 64, 4], [128 * 64, 2], [1, 64]]
            )
            nc.sync.dma_start(xt, src)

            # transpose: px[p,q,:] partitions 0-63 = feats of block 2q,
            # partitions 64-127 = feats of block 2q+1; free = 128 rows
            px = ps_x.tile([128, 4, 128], F32, tag="px")
            for q in range(4):
                nc.tensor.transpose(px[:, q], xt[:, q], ident)

            u1 = f1.tile([128, 4, 128], BF16, tag="u1")
            nc.vector.tensor_scalar(u1, px, -1.0, 1.0, op0=MAX, op1=MIN)
            sl1 = f1.tile([128, 4, 128], BF16, tag="sl1")
            nc.scalar.activation(sl1, px, SILU)
            c1 = f1.tile([128, 4, 128], BF16, tag="c1")
            nc.vector.tensor_scalar_max(c1, u1, 0.0)

            # L1: two concurrent 64-contraction row-group streams (A=even
            # blocks on partitions 0-63, B=odd blocks on 64-127)
            hA = ps_h.tile([128, 512], F32, tag="hA")
            hB = ps_h.tile([128, 512], F32, tag="hB")
            for c, ft in enumerate([sl1, u1, c1]):
                nc.tensor.matmul(hA, w1[0:64, c], ft[0:64], start=(c == 0), stop=(c == 2))
                nc.tensor.matmul(hB, w1[64:128, c], ft[64:128], start=(c == 0), stop=(c == 2))

            # L2: bias init via K=1 ones-matmul (sets has_written on the whole
            # bank so the 24 block matmuls accumulate with start=False).
            # Issued before the L2 features so PE has early work.
            po = ps_o.tile([128, 8, 64], F32, tag="po")
            nc.tensor.matmul(po, ones, b2r, start=True, stop=False)

            # L2 feature maps, merged A|B tiles: free 0-511 = A (even blocks),
            # 512-1023 = B (odd blocks)
            sl2 = f2.tile([128, 1024], BF16, tag="sl2")
            u2 = f2.tile([128, 1024], BF16, tag="u2")
            c2 = f2.tile([128, 1024], BF16, tag="c2")
            for half, h in enumerate([hA, hB]):
                sl = slice(half * 512, half * 512 + 512)
                nc.scalar.activation(sl2[:, sl], h, SILU, bias=b1)
                nc.vector.tensor_scalar(u2[:, sl], h, s1, s2, op0=MAX, op1=MIN)
            nc.vector.tensor_scalar_max(c2, u2, nb1)

            for g in range(8):
                off = (g % 2) * 512 + (g // 2) * 128
                for c, ft in enumerate([sl2, u2, c2]):
                    nc.tensor.matmul(
                        po[:, g],
                        ft[:, off : off + 128],
                        w2[:, c],
                        start=False,
                        stop=(g == 7 and c == 2),
                    )

            ot = osb.tile([128, 8, 64], F32, tag="ot")
            nc.scalar.copy(ot, po)
            dst = bass.AP(outd, base * 64, [[64, 128], [128 * 64, 8], [1, 64]])
            nc.sync.dma_start(dst, ot)

    nc.compile()
    return nc


def _get_nc(rows):
    if rows not in _nc_cache:
        _nc_cache[rows] = _build(rows)
    return _nc_cache[rows]


def kernel(x, cp0, bw0, sw0, imp0, cp1, bw1, sw1, imp1, _trace=False, _trace_kwargs=None):
    x = np.ascontiguousarray(np.asarray(x, dtype=np.float32))
    consts = _prep_consts(
        *[np.asarray(a, dtype=np.float32) for a in (cp0, bw0, sw0, imp0, cp1, bw1, sw1, imp1)]
    )
    rows = x.shape[0] // N_CORES
    nc = _get_nc(rows)
    in_maps = []
    for i in range(N_CORES):
        m = dict(consts)
        m["x"] = x[i * rows : (i + 1) * rows]
        in_maps.append(m)
    res = run_bass_kernel_spmd(
        nc, in_maps, list(range(N_CORES)), trace=_trace, **(_trace_kwargs or {})
    )
    out = np.concatenate([res.results[i]["out"] for i in range(N_CORES)], axis=0)
    if _trace:
        return out, res
    return out


# revision 13
# speedup vs baseline: 4.6425x; 1.1129x over previous
"""Fused 2-layer KAN for Trainium2, data-parallel across 8 NeuronCores.

Math: with G=3 grid points the spline basis is piecewise-linear in x, so each
KAN layer collapses to a small dense matmul over 3 cheap feature maps:

    out = bias + silu(x) @ Wb + u @ P1 + C @ (P2 - P1)
      u = clip(x, -1, 1),  C = max(u, 0)
      Wb = imp*bw;  T = imp*sw*cp;  P1 = T@(bv1-bv0);  P2 = T@(bv2-bv1)
      bias_j = sum_i T[i,j,:] @ bv1

All K=5 spline control points fold into P1/P2/bias on the host (O(I*J*K) work).
The device runs, per 1024-row macro-tile:
  DMA in -> PE transpose x to feature-major -> {silu, clip} feature maps
  -> L1 matmul (bf16, N=512, two 64-contraction row-group streams)
  -> L2 feature maps from PSUM -> L2 matmul (stationary features, N=64)
  -> bias via K=1 ones-matmul PSUM init -> copy to SBUF -> DMA out.
"""

import os
import sys
from contextlib import ExitStack

import numpy as np
import ml_dtypes

for _p in ("/opt/trn_rl_repo",):
    if _p not in sys.path and os.path.isdir(_p):
        sys.path.insert(0, _p)

import concourse.bass as bass
import concourse.tile as tile
from concourse import bacc, mybir
from concourse.bass_utils import run_bass_kernel_spmd
from concourse.masks import make_identity

F32 = mybir.dt.float32
BF16 = mybir.dt.bfloat16
BF = ml_dtypes.bfloat16

N_CORES = 8
D0, D1, D2 = 64, 128, 64
K, DEG, G, LO, HI = 5, 3, 3, -1.0, 1.0
MACRO = 1024  # batch rows per device macro-iteration

_nc_cache = {}


def _basis_table():
    knots = np.linspace(LO - DEG * 0.1, HI + DEG * 0.1, K + DEG + 1)
    grid = np.linspace(LO, HI, G)
    bv = np.zeros((G, K), dtype=np.float32)
    for i in range(K):
        center = (knots[i + DEG // 2] + knots[i + DEG // 2 + 1]) / 2.0
        width = (knots[i + DEG + 1] - knots[i]) / 2.0
        bv[:, i] = np.exp(-(((grid - center) / width) ** 2))
    bv = bv / (bv.sum(axis=1, keepdims=True) + 1e-6)
    return bv


def _prep_consts(cp0, bw0, sw0, imp0, cp1, bw1, sw1, imp1):
    f8 = np.float64
    bv = _basis_table().astype(f8)
    d1, d2 = bv[1] - bv[0], bv[2] - bv[1]

    def fold(cp, bw, sw, imp):
        T = imp.astype(f8)[:, :, None] * sw.astype(f8)[:, :, None] * cp.astype(f8)
        Wb = imp.astype(f8) * bw.astype(f8)
        return Wb, T @ d1, T @ d2, (T @ bv[1]).sum(axis=0)

    Wb0, P10, P20, b1 = fold(cp0, bw0, sw0, imp0)
    Wb1, P11, P21, b2 = fold(cp1, bw1, sw1, imp1)
    bias2_eff = b2 + b1 @ P21

    w1 = np.stack([Wb0, P10, P20 - P10], axis=0)  # [3, 64, 128] lhsT chunks
    w1 = np.concatenate([w1, w1], axis=1)  # duplicate rows for partitions 64-127
    w1 = np.ascontiguousarray(w1.transpose(1, 0, 2)).reshape(128, 384)
    w2 = np.stack([Wb1, P11, P21 - P11], axis=0)  # [3, 128, 64] rhs chunks
    w2 = np.ascontiguousarray(w2.transpose(1, 0, 2)).reshape(128, 192)

    return {
        "wpk": np.concatenate([w1, w2], axis=1).astype(BF),  # [128, 576]
        "spk": np.stack(
            [b1, -1.0 - b1, 1.0 - b1, -b1], axis=1
        ).astype(np.float32),  # [128, 4] = b1|s1|s2|nb1
        "b2row": np.tile(bias2_eff, 8).astype(BF).reshape(1, 512),
    }


def _build(rows):
    assert rows % MACRO == 0
    nc = bacc.Bacc(
        "TRN2",
        target_bir_lowering=False,
        debug=False,
        enable_asserts=False,
        num_devices=N_CORES,
    )
    xd = nc.dram_tensor("x", [rows, D0], F32, kind="ExternalInput")
    wpkd = nc.dram_tensor("wpk", [128, 576], BF16, kind="ExternalInput")
    spkd = nc.dram_tensor("spk", [128, 4], F32, kind="ExternalInput")
    b2d = nc.dram_tensor("b2row", [1, 512], BF16, kind="ExternalInput")
    outd = nc.dram_tensor("out", [rows, D2], F32, kind="ExternalOutput")

    n_macro = rows // MACRO
    MAX, MIN = mybir.AluOpType.max, mybir.AluOpType.min
    SILU = mybir.ActivationFunctionType.Silu

    with tile.TileContext(nc) as tc, ExitStack() as ctx:
        consts = ctx.enter_context(tc.tile_pool(name="consts", bufs=1))
        xin = ctx.enter_context(tc.tile_pool(name="xin", bufs=4))
        f1 = ctx.enter_context(tc.tile_pool(name="f1", bufs=3))
        f2 = ctx.enter_context(tc.tile_pool(name="f2", bufs=3))
        osb = ctx.enter_context(tc.tile_pool(name="osb", bufs=3))
        ps_x = ctx.enter_context(tc.tile_pool(name="ps_x", bufs=2, space="PSUM"))
        ps_h = ctx.enter_context(tc.tile_pool(name="ps_h", bufs=2, space="PSUM"))
        ps_o = ctx.enter_context(tc.tile_pool(name="ps_o", bufs=2, space="PSUM"))

        ident = consts.tile([128, 128], BF16)
        make_identity(nc, ident)
        ones = consts.tile([1, 128], BF16)
        nc.vector.memset(ones, 1.0)
        wpk = consts.tile([128, 576], BF16)
        nc.sync.dma_start(wpk, wpkd.ap())
        spk = consts.tile([128, 4], F32)
        nc.sync.dma_start(spk, spkd.ap())
        b2r = consts.tile([1, 512], BF16)
        nc.sync.dma_start(b2r, b2d.ap())
        b1, s1, s2, nb1 = (spk[:, i : i + 1] for i in range(4))
        w1c = [wpk[:, c * 128 : (c + 1) * 128] for c in range(3)]
        w2c = [wpk[:, 384 + c * 64 : 384 + (c + 1) * 64] for c in range(3)]

        for m in range(n_macro):
            base = m * MACRO
            # x[base + (2q+j)*128 + p, f] -> xt[p, q, j, f], cast to bf16 (SWDGE)
            xt = xin.tile([128, 4, 2, 64], BF16, tag="xt")
            src = bass.AP(
                xd, base * 64, [[64, 128], [2 * 128 * 64, 4], [128 * 64, 2], [1, 64]]
            )
            nc.gpsimd.dma_start(xt, src)

            # transpose: px[p,q,:] partitions 0-63 = feats of block 2q,
            # partitions 64-127 = feats of block 2q+1; free = 128 rows
            px = ps_x.tile([128, 4, 128], BF16, tag="px")
            for q in range(4):
                nc.tensor.transpose(px[:, q], xt[:, q], ident)

            u1 = f1.tile([128, 4, 128], BF16, tag="u1")
            nc.vector.tensor_scalar(u1, px, -1.0, 1.0, op0=MAX, op1=MIN)
            sl1 = f1.tile([128, 4, 128], BF16, tag="sl1")
            nc.scalar.activation(sl1, px, SILU)
            c1 = f1.tile([128, 4, 128], BF16, tag="c1")
            nc.vector.tensor_scalar_max(c1, u1, 0.0)

            # L1: two concurrent 64-contraction row-group streams (A=even
            # blocks on partitions 0-63, B=odd blocks on 64-127)
            hA = ps_h.tile([128, 512], F32, tag="hA")
            hB = ps_h.tile([128, 512], F32, tag="hB")
            for c, ft in enumerate([sl1, u1, c1]):
                nc.tensor.matmul(hA, w1c[c][0:64], ft[0:64], start=(c == 0), stop=(c == 2))
                nc.tensor.matmul(hB, w1c[c][64:128], ft[64:128], start=(c == 0), stop=(c == 2))

            # L2: bias init via K=1 ones-matmul (sets has_written on the whole
            # bank so the 24 block matmuls accumulate with start=False).
            # Issued before the L2 features so PE has early work.
            po = ps_o.tile([128, 8, 64], F32, tag="po")
            nc.tensor.matmul(po, ones, b2r, start=True, stop=False)

            # L2 feature maps, merged A|B tiles: free 0-511 = A (even blocks),
            # 512-1023 = B (odd blocks)
            sl2 = f2.tile([128, 1024], BF16, tag="sl2")
            u2 = f2.tile([128, 1024], BF16, tag="u2")
            c2 = f2.tile([128, 1024], BF16, tag="c2")
            for half, h in enumerate([hA, hB]):
                sl = slice(half * 512, half * 512 + 512)
                nc.scalar.activation(sl2[:, sl], h, SILU, bias=b1)
                nc.vector.tensor_scalar(u2[:, sl], h, s1, s2, op0=MAX, op1=MIN)
            nc.vector.tensor_scalar_max(c2, u2, nb1)

            for g in range(8):
                off = (g % 2) * 512 + (g // 2) * 128
                for c, ft in enumerate([sl2, u2, c2]):
                    nc.tensor.matmul(
                        po[:, g],
                        ft[:, off : off + 128],
                        w2c[c],
                        start=False,
                        stop=(g == 7 and c == 2),
                    )

            ot = osb.tile([128, 8, 64], F32, tag="ot")
            nc.scalar.copy(ot, po)
            dst = bass.AP(outd, base * 64, [[64, 128], [128 * 64, 8], [1, 64]])
            nc.sync.dma_start(dst, ot)

    nc.compile()
    return nc


def _get_nc(rows):
    if rows not in _nc_cache:
        _nc_cache[rows] = _build(rows)
    return _nc_cache[rows]


def kernel(x, cp0, bw0, sw0, imp0, cp1, bw1, sw1, imp1, _trace=False, _trace_kwargs=None):
    x = np.ascontiguousarray(np.asarray(x, dtype=np.float32))
    consts = _prep_consts(
        *[np.asarray(a, dtype=np.float32) for a in (cp0, bw0, sw0, imp0, cp1, bw1, sw1, imp1)]
    )
    rows = x.shape[0] // N_CORES
    nc = _get_nc(rows)
    in_maps = []
    for i in range(N_CORES):
        m = dict(consts)
        m["x"] = x[i * rows : (i + 1) * rows]
        in_maps.append(m)
    res = run_bass_kernel_spmd(
        nc, in_maps, list(range(N_CORES)), trace=_trace, **(_trace_kwargs or {})
    )
    out = np.concatenate([res.results[i]["out"] for i in range(N_CORES)], axis=0)
    if _trace:
        return out, res
    return out


# revision 16
# speedup vs baseline: 5.1913x; 1.1182x over previous
"""Fused 2-layer KAN for Trainium2, data-parallel across 8 NeuronCores.

Math: with G=3 grid points the spline basis is piecewise-linear in x, so each
KAN layer collapses to a small dense matmul over 3 cheap feature maps:

    out = bias + silu(x) @ Wb + u @ P1 + C @ (P2 - P1)
      u = clip(x, -1, 1),  C = max(u, 0)
      Wb = imp*bw;  T = imp*sw*cp;  P1 = T@(bv1-bv0);  P2 = T@(bv2-bv1)
      bias_j = sum_i T[i,j,:] @ bv1

All K=5 spline control points fold into P1/P2/bias on the host (O(I*J*K) work).
The device runs, per 1024-row macro-tile:
  DMA in -> PE transpose x to feature-major -> {silu, clip} feature maps
  -> L1 matmul (bf16, N=512, two 64-contraction row-group streams)
  -> L2 feature maps from PSUM -> L2 matmul (stationary features, N=64)
  -> bias via K=1 ones-matmul PSUM init -> copy to SBUF -> DMA out.
"""

import os
import sys
from contextlib import ExitStack

import numpy as np
import ml_dtypes

for _p in ("/opt/trn_rl_repo",):
    if _p not in sys.path and os.path.isdir(_p):
        sys.path.insert(0, _p)

import concourse.bass as bass
import concourse.tile as tile
from concourse import bacc, mybir
from concourse.bass_utils import run_bass_kernel_spmd
from concourse.masks import make_identity

F32 = mybir.dt.float32
BF16 = mybir.dt.bfloat16
BF = ml_dtypes.bfloat16

N_CORES = 8
D0, D1, D2 = 64, 128, 64
K, DEG, G, LO, HI = 5, 3, 3, -1.0, 1.0
MACRO = 1024  # batch rows per device macro-iteration

_nc_cache = {}


def _basis_table():
    knots = np.linspace(LO - DEG * 0.1, HI + DEG * 0.1, K + DEG + 1)
    grid = np.linspace(LO, HI, G)
    bv = np.zeros((G, K), dtype=np.float32)
    for i in range(K):
        center = (knots[i + DEG // 2] + knots[i + DEG // 2 + 1]) / 2.0
        width = (knots[i + DEG + 1] - knots[i]) / 2.0
        bv[:, i] = np.exp(-(((grid - center) / width) ** 2))
    bv = bv / (bv.sum(axis=1, keepdims=True) + 1e-6)
    return bv


def _prep_consts(cp0, bw0, sw0, imp0, cp1, bw1, sw1, imp1):
    f8 = np.float64
    bv = _basis_table().astype(f8)
    d1, d2 = bv[1] - bv[0], bv[2] - bv[1]

    def fold(cp, bw, sw, imp):
        T = imp.astype(f8)[:, :, None] * sw.astype(f8)[:, :, None] * cp.astype(f8)
        Wb = imp.astype(f8) * bw.astype(f8)
        return Wb, T @ d1, T @ d2, (T @ bv[1]).sum(axis=0)

    Wb0, P10, P20, b1 = fold(cp0, bw0, sw0, imp0)
    Wb1, P11, P21, b2 = fold(cp1, bw1, sw1, imp1)
    bias2_eff = b2 + b1 @ P21

    w1 = np.stack([Wb0, P10, P20 - P10], axis=0)  # [3, 64, 128] lhsT chunks
    w1 = np.concatenate([w1, w1], axis=1)  # duplicate rows for partitions 64-127
    w1 = np.ascontiguousarray(w1.transpose(1, 0, 2)).reshape(128, 384)
    w2 = np.stack([Wb1, P11, P21 - P11], axis=0)  # [3, 128, 64] rhs chunks
    w2 = np.ascontiguousarray(w2.transpose(1, 0, 2)).reshape(128, 192)

    return {
        "wpk": np.concatenate([w1, w2], axis=1).astype(BF),  # [128, 576]
        "spk": np.stack(
            [b1, -1.0 - b1, 1.0 - b1, -b1], axis=1
        ).astype(np.float32),  # [128, 4] = b1|s1|s2|nb1
        "b2row": np.tile(bias2_eff, 8).astype(BF).reshape(1, 512),
    }


def _build(rows):
    assert rows % MACRO == 0
    nc = bacc.Bacc(
        "TRN2",
        target_bir_lowering=False,
        debug=False,
        enable_asserts=False,
        num_devices=N_CORES,
    )
    xd = nc.dram_tensor("x", [rows, D0], F32, kind="ExternalInput")
    wpkd = nc.dram_tensor("wpk", [128, 576], BF16, kind="ExternalInput")
    spkd = nc.dram_tensor("spk", [128, 4], F32, kind="ExternalInput")
    b2d = nc.dram_tensor("b2row", [1, 512], BF16, kind="ExternalInput")
    outd = nc.dram_tensor("out", [rows, D2], F32, kind="ExternalOutput")

    n_macro = rows // MACRO
    MAX, MIN = mybir.AluOpType.max, mybir.AluOpType.min
    SILU = mybir.ActivationFunctionType.Silu

    with tile.TileContext(nc) as tc, ExitStack() as ctx:
        consts = ctx.enter_context(tc.tile_pool(name="consts", bufs=1))
        xin = ctx.enter_context(tc.tile_pool(name="xin", bufs=4))
        f1 = ctx.enter_context(tc.tile_pool(name="f1", bufs=3))
        f2 = ctx.enter_context(tc.tile_pool(name="f2", bufs=3))
        osb = ctx.enter_context(tc.tile_pool(name="osb", bufs=3))
        ps_x = ctx.enter_context(tc.tile_pool(name="ps_x", bufs=2, space="PSUM"))
        ps_h = ctx.enter_context(tc.tile_pool(name="ps_h", bufs=2, space="PSUM"))
        ps_o = ctx.enter_context(tc.tile_pool(name="ps_o", bufs=2, space="PSUM"))

        ident = consts.tile([128, 128], BF16)
        make_identity(nc, ident)
        ones = consts.tile([1, 128], BF16)
        nc.vector.memset(ones, 1.0)
        wpk = consts.tile([128, 576], BF16)
        nc.sync.dma_start(wpk, wpkd.ap())
        spk = consts.tile([128, 4], F32)
        nc.sync.dma_start(spk, spkd.ap())
        b2r = consts.tile([1, 512], BF16)
        nc.sync.dma_start(b2r, b2d.ap())
        b1, s1, s2, nb1 = (spk[:, i : i + 1] for i in range(4))
        w1c = [wpk[:, c * 128 : (c + 1) * 128] for c in range(3)]
        w2c = [wpk[:, 384 + c * 64 : 384 + (c + 1) * 64] for c in range(3)]

        # PE pre-warm: ~30 dummy matmuls while DMAs land, so the HAM clock
        # gate opens (1.2 -> 2.4 GHz) before the first real matmul issues.
        warm = ps_o.tile([128, 8, 64], F32, tag="po")
        for _ in range(30):
            nc.tensor.matmul(warm[:, 0:2], ident, ident, start=True, stop=True)

        for m in range(n_macro):
            base = m * MACRO
            # x[base + (2q+j)*128 + p, f] -> xt[p, q, j, f], cast to bf16 (SWDGE)
            xt = xin.tile([128, 4, 2, 64], BF16, tag="xt")
            src = bass.AP(
                xd, base * 64, [[64, 128], [2 * 128 * 64, 4], [128 * 64, 2], [1, 64]]
            )
            nc.gpsimd.dma_start(xt, src)

            # transpose: px[p,q,:] partitions 0-63 = feats of block 2q,
            # partitions 64-127 = feats of block 2q+1; free = 128 rows
            px = ps_x.tile([128, 4, 128], BF16, tag="px")
            for q in range(4):
                nc.tensor.transpose(px[:, q], xt[:, q], ident)

            # u1 first: then c1 (DVE, from SBUF) overlaps sl1 (ACT, from PSUM)
            u1 = f1.tile([128, 4, 128], BF16, tag="u1")
            nc.vector.tensor_scalar(u1, px, -1.0, 1.0, op0=MAX, op1=MIN)
            sl1 = f1.tile([128, 4, 128], BF16, tag="sl1")
            nc.scalar.activation(sl1, px, SILU)
            c1 = f1.tile([128, 4, 128], BF16, tag="c1")
            nc.vector.tensor_scalar_max(c1, u1, 0.0)

            # L1: two concurrent 64-contraction row-group streams (A=even
            # blocks on partitions 0-63, B=odd blocks on 64-127)
            hA = ps_h.tile([128, 512], F32, tag="hA")
            hB = ps_h.tile([128, 512], F32, tag="hB")
            for c, ft in enumerate([sl1, u1, c1]):
                nc.tensor.matmul(hA, w1c[c][0:64], ft[0:64], start=(c == 0), stop=(c == 2))
                nc.tensor.matmul(hB, w1c[c][64:128], ft[64:128], start=(c == 0), stop=(c == 2))

            # L2: bias init via K=1 ones-matmul (sets has_written on the whole
            # bank so the 24 block matmuls accumulate with start=False).
            # Issued before the L2 features so PE has early work.
            po = ps_o.tile([128, 8, 64], F32, tag="po")
            nc.tensor.matmul(po, ones, b2r, start=True, stop=False)

            # L2 feature maps, merged A|B tiles: free 0-511 = A (even blocks),
            # 512-1023 = B (odd blocks)
            # Cross the A/B banks between ACT and DVE so the two engines never
            # contend on the same PSUM bank (Tile serializes same-bank pairs).
            sl2 = f2.tile([128, 1024], BF16, tag="sl2")
            u2 = f2.tile([128, 1024], BF16, tag="u2")
            c2 = f2.tile([128, 1024], BF16, tag="c2")
            sA, sB = slice(0, 512), slice(512, 1024)
            nc.scalar.activation(sl2[:, sA], hA, SILU, bias=b1)
            nc.vector.tensor_scalar(u2[:, sB], hB, s1, s2, op0=MAX, op1=MIN)
            nc.scalar.activation(sl2[:, sB], hB, SILU, bias=b1)
            nc.vector.tensor_scalar(u2[:, sA], hA, s1, s2, op0=MAX, op1=MIN)
            nc.vector.tensor_scalar_max(c2, u2, nb1)

            for g in range(8):
                off = (g % 2) * 512 + (g // 2) * 128
                for c, ft in enumerate([sl2, u2, c2]):
                    nc.tensor.matmul(
                        po[:, g],
                        ft[:, off : off + 128],
                        w2c[c],
                        start=False,
                        stop=(g == 7 and c == 2),
                    )

            ot = osb.tile([128, 8, 64], F32, tag="ot")
            nc.scalar.copy(ot, po)
            dst = bass.AP(outd, base * 64, [[64, 128], [128 * 64, 8], [1, 64]])
            nc.sync.dma_start(dst, ot)

    nc.compile()
    return nc


def _get_nc(rows):
    if rows not in _nc_cache:
        _nc_cache[rows] = _build(rows)
    return _nc_cache[rows]


def kernel(x, cp0, bw0, sw0, imp0, cp1, bw1, sw1, imp1, _trace=False, _trace_kwargs=None):
    x = np.ascontiguousarray(np.asarray(x, dtype=np.float32))
    consts = _prep_consts(
        *[np.asarray(a, dtype=np.float32) for a in (cp0, bw0, sw0, imp0, cp1, bw1, sw1, imp1)]
    )
    rows = x.shape[0] // N_CORES
    nc = _get_nc(rows)
    in_maps = []
    for i in range(N_CORES):
        m = dict(consts)
        m["x"] = x[i * rows : (i + 1) * rows]
        in_maps.append(m)
    res = run_bass_kernel_spmd(
        nc, in_maps, list(range(N_CORES)), trace=_trace, **(_trace_kwargs or {})
    )
    out = np.concatenate([res.results[i]["out"] for i in range(N_CORES)], axis=0)
    if _trace:
        return out, res
    return out


# revision 19
# speedup vs baseline: 5.2171x; 1.0050x over previous
"""Fused 2-layer KAN for Trainium2, data-parallel across 8 NeuronCores.

Math: with G=3 grid points the spline basis is piecewise-linear in x, so each
KAN layer collapses to a small dense matmul over 3 cheap feature maps:

    out = bias + silu(x) @ Wb + u @ P1 + C @ (P2 - P1)
      u = clip(x, -1, 1),  C = max(u, 0)
      Wb = imp*bw;  T = imp*sw*cp;  P1 = T@(bv1-bv0);  P2 = T@(bv2-bv1)
      bias_j = sum_i T[i,j,:] @ bv1

All K=5 spline control points fold into P1/P2/bias on the host (O(I*J*K) work).
The device runs, per 1024-row macro-tile:
  DMA in -> PE transpose x to feature-major -> {silu, clip} feature maps
  -> L1 matmul (bf16, N=512, two 64-contraction row-group streams)
  -> L2 feature maps from PSUM -> L2 matmul (stationary features, N=64)
  -> bias via K=1 ones-matmul PSUM init -> copy to SBUF -> DMA out.
"""

import os
import sys
from contextlib import ExitStack

import numpy as np
import ml_dtypes

for _p in ("/opt/trn_rl_repo",):
    if _p not in sys.path and os.path.isdir(_p):
        sys.path.insert(0, _p)

import concourse.bass as bass
import concourse.tile as tile
from concourse import bacc, mybir
from concourse.bass_utils import run_bass_kernel_spmd
from concourse.masks import make_identity

F32 = mybir.dt.float32
BF16 = mybir.dt.bfloat16
BF = ml_dtypes.bfloat16

N_CORES = 8
D0, D1, D2 = 64, 128, 64
K, DEG, G, LO, HI = 5, 3, 3, -1.0, 1.0
MACRO = 1024  # batch rows per device macro-iteration

_nc_cache = {}


def _basis_table():
    knots = np.linspace(LO - DEG * 0.1, HI + DEG * 0.1, K + DEG + 1)
    grid = np.linspace(LO, HI, G)
    bv = np.zeros((G, K), dtype=np.float32)
    for i in range(K):
        center = (knots[i + DEG // 2] + knots[i + DEG // 2 + 1]) / 2.0
        width = (knots[i + DEG + 1] - knots[i]) / 2.0
        bv[:, i] = np.exp(-(((grid - center) / width) ** 2))
    bv = bv / (bv.sum(axis=1, keepdims=True) + 1e-6)
    return bv


def _prep_consts(cp0, bw0, sw0, imp0, cp1, bw1, sw1, imp1):
    f8 = np.float64
    bv = _basis_table().astype(f8)
    d1, d2 = bv[1] - bv[0], bv[2] - bv[1]

    def fold(cp, bw, sw, imp):
        T = imp.astype(f8)[:, :, None] * sw.astype(f8)[:, :, None] * cp.astype(f8)
        Wb = imp.astype(f8) * bw.astype(f8)
        return Wb, T @ d1, T @ d2, (T @ bv[1]).sum(axis=0)

    Wb0, P10, P20, b1 = fold(cp0, bw0, sw0, imp0)
    Wb1, P11, P21, b2 = fold(cp1, bw1, sw1, imp1)
    bias2_eff = b2 + b1 @ P21

    w1 = np.stack([Wb0, P10, P20 - P10], axis=0)  # [3, 64, 128] lhsT chunks
    w1 = np.concatenate([w1, w1], axis=1)  # duplicate rows for partitions 64-127
    w1 = np.ascontiguousarray(w1.transpose(1, 0, 2)).reshape(128, 384)
    w2 = np.stack([Wb1, P11, P21 - P11], axis=0)  # [3, 128, 64] rhs chunks
    w2 = np.ascontiguousarray(w2.transpose(1, 0, 2)).reshape(128, 192)

    return {
        "wpk": np.concatenate([w1, w2], axis=1).astype(BF),  # [128, 576]
        "spk": np.stack(
            [b1, -1.0 - b1, 1.0 - b1, -b1], axis=1
        ).astype(np.float32),  # [128, 4] = b1|s1|s2|nb1
        "b2row": np.tile(bias2_eff, 8).astype(BF).reshape(1, 512),
    }


def _build(rows):
    assert rows % MACRO == 0
    nc = bacc.Bacc(
        "TRN2",
        target_bir_lowering=False,
        debug=False,
        enable_asserts=False,
        num_devices=N_CORES,
    )
    xd = nc.dram_tensor("x", [rows, D0], F32, kind="ExternalInput")
    wpkd = nc.dram_tensor("wpk", [128, 576], BF16, kind="ExternalInput")
    spkd = nc.dram_tensor("spk", [128, 4], F32, kind="ExternalInput")
    b2d = nc.dram_tensor("b2row", [1, 512], BF16, kind="ExternalInput")
    outd = nc.dram_tensor("out", [rows, D2], F32, kind="ExternalOutput")

    n_macro = rows // MACRO
    MAX, MIN = mybir.AluOpType.max, mybir.AluOpType.min
    SILU = mybir.ActivationFunctionType.Silu

    with tile.TileContext(nc) as tc, ExitStack() as ctx:
        consts = ctx.enter_context(tc.tile_pool(name="consts", bufs=1))
        xin = ctx.enter_context(tc.tile_pool(name="xin", bufs=4))
        f1 = ctx.enter_context(tc.tile_pool(name="f1", bufs=3))
        f2 = ctx.enter_context(tc.tile_pool(name="f2", bufs=3))
        osb = ctx.enter_context(tc.tile_pool(name="osb", bufs=3))
        ps_x = ctx.enter_context(tc.tile_pool(name="ps_x", bufs=2, space="PSUM"))
        ps_h = ctx.enter_context(tc.tile_pool(name="ps_h", bufs=2, space="PSUM"))
        ps_o = ctx.enter_context(tc.tile_pool(name="ps_o", bufs=2, space="PSUM"))

        ident = consts.tile([128, 128], BF16)
        make_identity(nc, ident)
        ones = consts.tile([1, 128], BF16)
        nc.vector.memset(ones, 1.0)
        wpk = consts.tile([128, 576], BF16)
        nc.sync.dma_start(wpk, wpkd.ap())
        spk = consts.tile([128, 4], F32)
        nc.sync.dma_start(spk, spkd.ap())
        b2r = consts.tile([1, 512], BF16)
        nc.sync.dma_start(b2r, b2d.ap())
        b1, s1, s2, nb1 = (spk[:, i : i + 1] for i in range(4))
        w1c = [wpk[:, c * 128 : (c + 1) * 128] for c in range(3)]
        w2c = [wpk[:, 384 + c * 64 : 384 + (c + 1) * 64] for c in range(3)]

        # PE pre-warm: ~30 dummy matmuls while DMAs land, so the HAM clock
        # gate opens (1.2 -> 2.4 GHz) before the first real matmul issues.
        warm = ps_o.tile([128, 8, 64], F32, tag="po")
        for _ in range(48):
            nc.tensor.matmul(warm[:, 0:2], ident, ident, start=True, stop=True)

        for m in range(n_macro):
            base = m * MACRO
            # x[base + (2q+j)*128 + p, f] -> xt[p, q, j, f], cast to bf16 (SWDGE)
            xt = xin.tile([128, 4, 2, 64], BF16, tag="xt")
            src = bass.AP(
                xd, base * 64, [[64, 128], [2 * 128 * 64, 4], [128 * 64, 2], [1, 64]]
            )
            nc.gpsimd.dma_start(xt, src)

            # transpose: px[p,q,:] partitions 0-63 = feats of block 2q,
            # partitions 64-127 = feats of block 2q+1; free = 128 rows
            px = ps_x.tile([128, 4, 128], BF16, tag="px")
            for q in range(4):
                nc.tensor.transpose(px[:, q], xt[:, q], ident)

            # u1 first: then c1 (DVE, from SBUF) overlaps sl1 (ACT, from PSUM)
            u1 = f1.tile([128, 4, 128], BF16, tag="u1")
            nc.vector.tensor_scalar(u1, px, -1.0, 1.0, op0=MAX, op1=MIN)
            sl1 = f1.tile([128, 4, 128], BF16, tag="sl1")
            nc.scalar.activation(sl1, px, SILU)
            c1 = f1.tile([128, 4, 128], BF16, tag="c1")
            nc.vector.tensor_scalar_max(c1, u1, 0.0)

            # L1: two concurrent 64-contraction row-group streams (A=even
            # blocks on partitions 0-63, B=odd blocks on 64-127)
            hA = ps_h.tile([128, 512], F32, tag="hA")
            hB = ps_h.tile([128, 512], F32, tag="hB")
            # chunk order = feature readiness order (u1 -> sl1 -> c1)
            for i, (c, ft) in enumerate([(1, u1), (0, sl1), (2, c1)]):
                nc.tensor.matmul(hA, w1c[c][0:64], ft[0:64], start=(i == 0), stop=(i == 2))
                nc.tensor.matmul(hB, w1c[c][64:128], ft[64:128], start=(i == 0), stop=(i == 2))

            # L2: bias init via K=1 ones-matmul (sets has_written on the whole
            # bank so the 24 block matmuls accumulate with start=False).
            # Issued before the L2 features so PE has early work.
            po = ps_o.tile([128, 8, 64], F32, tag="po")
            nc.tensor.matmul(po, ones, b2r, start=True, stop=False)

            # L2 feature maps, merged A|B tiles: free 0-511 = A (even blocks),
            # 512-1023 = B (odd blocks)
            # Cross the A/B banks between ACT and DVE so the two engines never
            # contend on the same PSUM bank (Tile serializes same-bank pairs).
            sl2 = f2.tile([128, 1024], BF16, tag="sl2")
            u2 = f2.tile([128, 1024], BF16, tag="u2")
            c2 = f2.tile([128, 1024], BF16, tag="c2")
            sA, sB = slice(0, 512), slice(512, 1024)
            nc.scalar.activation(sl2[:, sA], hA, SILU, bias=b1)
            nc.vector.tensor_scalar(u2[:, sB], hB, s1, s2, op0=MAX, op1=MIN)
            nc.scalar.activation(sl2[:, sB], hB, SILU, bias=b1)
            nc.vector.tensor_scalar(u2[:, sA], hA, s1, s2, op0=MAX, op1=MIN)
            nc.vector.tensor_scalar_max(c2, u2, nb1)

            # chunk-major, each chunk's blocks ordered by which half is ready
            # first (sl2 fills A then B; u2 fills B then A)
            plan = [
                (0, sl2, (0, 2, 4, 6, 1, 3, 5, 7)),
                (1, u2, (1, 3, 5, 7, 0, 2, 4, 6)),
                (2, c2, (0, 1, 2, 3, 4, 5, 6, 7)),
            ]
            for ci, (c, ft, order) in enumerate(plan):
                for gi, g in enumerate(order):
                    off = (g % 2) * 512 + (g // 2) * 128
                    nc.tensor.matmul(
                        po[:, g],
                        ft[:, off : off + 128],
                        w2c[c],
                        start=False,
                        stop=(ci == 2 and gi == 7),
                    )

            ot = osb.tile([128, 8, 64], F32, tag="ot")
            nc.scalar.copy(ot, po)
            dst = bass.AP(outd, base * 64, [[64, 128], [128 * 64, 8], [1, 64]])
            nc.sync.dma_start(dst, ot)

    nc.compile()
    return nc


def _get_nc(rows):
    if rows not in _nc_cache:
        _nc_cache[rows] = _build(rows)
    return _nc_cache[rows]


def kernel(x, cp0, bw0, sw0, imp0, cp1, bw1, sw1, imp1, _trace=False, _trace_kwargs=None):
    x = np.ascontiguousarray(np.asarray(x, dtype=np.float32))
    consts = _prep_consts(
        *[np.asarray(a, dtype=np.float32) for a in (cp0, bw0, sw0, imp0, cp1, bw1, sw1, imp1)]
    )
    rows = x.shape[0] // N_CORES
    nc = _get_nc(rows)
    in_maps = []
    for i in range(N_CORES):
        m = dict(consts)
        m["x"] = x[i * rows : (i + 1) * rows]
        in_maps.append(m)
    res = run_bass_kernel_spmd(
        nc, in_maps, list(range(N_CORES)), trace=_trace, **(_trace_kwargs or {})
    )
    out = np.concatenate([res.results[i]["out"] for i in range(N_CORES)], axis=0)
    if _trace:
        return out, res
    return out
